# revision 47
# baseline (speedup 1.0000x reference)
"""Trainium2 Bass kernel for nn_NExpR_14903536517949 (embedding_lookup).

Reference computation per query point (b, n):
    hi = floor(gx/2), wi = floor(gy/2)                 (bin indices, 64x64 grid)
    params = function_map[b, hi, wi]                   (162 channels: Ps|Pc)
    lx = gx mod 2, ly = gy mod 2                       (local coords)
    out = sum_ij Ps_ij sin(lx xw_i + ly yw_j) + Pc_ij cos(lx xw_i + ly yw_j)

Host-side algebraic transforms:
  * out = Im sum_ij C_ij e^{i b_ij}, C = Ps + i Pc, b_ij = 2pi(fx_i lx + fy_j ly)
    (freqs in turns). Terms sharing a frequency pair merge; Hermitian pairs
    (f, -f) merge via C' = C_f - conj(C_{-f}). For the spec input (uniform
    basis 0.5 -> quarter-integer freqs with a doubled zero row) 81 terms
    collapse to NT = 40 amplitude/phase pairs: out = sum_t A_t sin(2pi(fx_t lx
    + fy_t ly + phi_t)).
  * Points are sorted by bin per batch and greedy-packed into 240 slots of
    <=128 points drawing from <=64 distinct bins each (the per-slot table
    carries exactly those bins' rows), so each point's bin row is fetched by
    a one-hot matmul instead of a per-point DMA gather. Output is
    un-permuted on host. Works for any coord distribution.

Device pipeline per slot (128 points), all data delivered by dense DMA
(no per-point gather):
  * PE: two matmuls with a shared fp8 lhsT [72, 128] = [8 coord-split rows |
    64 one-hot window rows] against the slot's f16 table column block:
      w-psum = basis + phi   (coord rows x freq rows + one-hot x phi rows)
      A-psum = amplitudes    (one-hot x A rows)
  * DVE: FRAC1 custom op m = w - round(w) (fp32 magic) over a PAIR of
    12-slot units sharing a 2-bank w-psum tile, fp16 out; SEGSUM custom op
    per-slot prefix sums of q*A against the unit's A-psum (totals at col
    NT-1). DVE is the bottleneck engine (~79% busy); segsums lag one pair
    behind sin so DVE never waits on ACT.
  * ACT: q = sin(SIN_SCALE * m) per pair.
  * Pool: copy per-slot totals into the result tile.
  * Inputs stream in unit-range chunks so compute starts ~5us in; outputs
    drain in two DMAs per batch.

Distribution: data-parallel over batch, 2 images per core, 8 cores.
"""

import math

import numpy as np
import ml_dtypes

import concourse.bass as bass
import concourse.mybir as mybir
import concourse.tile as tile
from concourse import bacc
from concourse.bass_utils import run_bass_kernel_spmd

import concourse.dve_ops as dve_ops
from concourse.dve_spec import C0, Spec, Src0, Src1, lower
from concourse.dve_uop import DveOpSpec

F32 = mybir.dt.float32
F16 = mybir.dt.float16
F8 = mybir.dt.float8e4
ALU = mybir.AluOpType
AFT = mybir.ActivationFunctionType

# Problem shape (hardcoded per spec)
B, H, W, C = 16, 64, 64, 162
N = 30000
NCORES = 8
BPC = B // NCORES            # batches per core = 2
DEG, MAXB, BAR = 8, 4.0, 2.0
L = DEG + 1                  # 9
NB = H * W                   # bins per batch = 4096
TWO_PI = 2.0 * math.pi
SIN_SCALE = 6.2831820        # slightly under 2*pi: |m*scale| < pi at m=+-0.5
RND_MAGIC = 1.5 * 2.0**23    # fp32 add-sub round-to-nearest trick

# Kernel layout constants
P = 128                      # points per slot (partitions)
S = 240                      # slots per batch
ND = P * S                   # 30720 padded points per batch
KC = 8                       # coord-split lhsT rows (4 per axis, x16 scales)
KW = 64                      # one-hot window rows (bins per window)
K = KC + KW                  # 72
US = 12                      # slots per pipeline unit (psum bank = 512 f32)
NU = S // US                 # 20 units per batch
F8NP = ml_dtypes.float8_e4m3fn


def _frac1_ref(in0, in1, s0, s1=0.0, imm2=0.0):
    w = np.asarray(in0, np.float32)
    r = (w + np.float32(s0)) - np.float32(s0)
    return w - r


def _register_frac1():
    """Custom DVE op: out = w - round(w) (fp32 magic-number rounding)."""
    if "FRAC1_ANT" in dve_ops._SUB_OPCODE_FOR_NAME:
        return next(op for op in dve_ops.OPS if op.name == "FRAC1_ANT")
    w = Src0
    spec = Spec(body=w - ((w + C0) - C0), reference=_frac1_ref)
    shas = {}
    for ver in ("v3", "v4"):
        d = DveOpSpec(name="FRAC1_ANT", opcode=0, uops=lower(spec, ver=ver),
                      rd1_en=False)
        shas[ver] = d.sha(ver)
    op = dve_ops.DveOp("FRAC1_ANT", spec, subdim=False, uops_sha=shas)
    dve_ops.OPS.append(op)
    dve_ops._SUB_OPCODE_FOR_NAME[op.name] = (
        dve_ops._CUSTOM_DVE_ROW_BASE + len(dve_ops.OPS) - 1
    )
    dve_ops.CUSTOM_DVE_SPECS[op.name] = op.spec
    return op


def _segsum_ref(in0, in1, s0, s1=0.0, imm2=0.0):
    a = np.asarray(in0, np.float32)
    b = np.asarray(in1, np.float32)
    return np.cumsum(a * b, axis=-1) * np.float32(s1)


def _register_segsum():
    """Hand-built 3-state uop FSM: per-page running prefix sums of
    Src0*Src1*s1 (page = innermost dim); each page's total lands at its
    last column."""
    import dataclasses as _dc
    from concourse.dve_uop import Trigger, OutPath, OutSel, AluInp, AluOp

    if "SEGSUM_ANT" in dve_ops._SUB_OPCODE_FOR_NAME:
        return next(op for op in dve_ops.OPS if op.name == "SEGSUM_ANT")

    def _build(ver):
        from concourse.dve_ops import TENSOR_TENSOR_REDUCE as TTR
        u0, u1 = [_dc.replace(u) for u in lower(TTR.spec, ver=ver)]
        acc_stage = next(
            i for i, dp in enumerate(u1.datapath_config)
            if dp.op == AluOp.ADD and dp.alu_src0 == AluInp.CURR_ALU_OUT
        )
        dp_seed = list(u0.datapath_config)
        dp_seed[acc_stage] = _dc.replace(
            dp_seed[acc_stage], op=AluOp.BYPASS,
            alu_src0=AluInp.PREV_ALU_OUT, alu_src1=AluInp.PREV_ALU_OUT,
        )
        wr = {**u1.out, OutPath.WR0_LO: OutSel.ALU_OUT,
              OutPath.WR0_HI: OutSel.ALU_OUT}
        wren = {**{p: 0 for p in OutPath}, OutPath.WR0_LO: 1,
                OutPath.WR0_HI: 1}
        init = _dc.replace(
            u0, datapath_config=dp_seed, out=wr, out_enable=wren,
            require_inp0=1, require_inp1=1,
            trigger=(Trigger.COUNT, Trigger.NONE, Trigger.NONE),
            next_uop=(1, 0, 0), repeat_count=1,
        )
        steady = _dc.replace(
            u1, out=wr, out_enable=wren,
            trigger=(Trigger.SRC_TENSOR_DONE, Trigger.SUB_DIM_DONE,
                     Trigger.NONE),
            next_uop=(0, 2, 0),
        )
        pageseed = _dc.replace(
            u0, datapath_config=dp_seed, out=wr, out_enable=wren,
            require_inp0=1, require_inp1=1,
            trigger=(Trigger.SRC_TENSOR_DONE, Trigger.SUB_DIM_DONE,
                     Trigger.COUNT),
            next_uop=(0, 2, 1), repeat_count=1,
        )
        for u in (init, steady, pageseed):
            u.validate(ver)
        return [init, steady, pageseed]

    class _HandDveOp(dve_ops.DveOp):
        def compile(self, ver):
            key = (self.name, ver)
            if (r := dve_ops._COMPILE_CACHE.get(key)) is not None:
                return r
            result = DveOpSpec(
                name=self.name,
                opcode=dve_ops.get_dve_sub_opcode(self.name),
                uops=_build(ver), rd1_en=True,
            )
            dve_ops._COMPILE_CACHE[key] = result
            return result

    spec = Spec(body=Src0 * Src1, reference=_segsum_ref)
    op = _HandDveOp("SEGSUM_ANT", spec, subdim=True, uops_sha={})
    dve_ops.OPS.append(op)
    dve_ops._SUB_OPCODE_FOR_NAME[op.name] = (
        dve_ops._CUSTOM_DVE_ROW_BASE + len(dve_ops.OPS) - 1
    )
    dve_ops.CUSTOM_DVE_SPECS[op.name] = op.spec
    return op


FRAC1 = _register_frac1()
SEGSUM = _register_segsum()


def build_bass(nt):
    """nt = number of merged terms (compile-time)."""
    assert US * nt <= 512, f"unit exceeds psum bank: {nt=}"
    ts = 2 * nt                  # table cols per window: [phi | A]
    nc = bacc.Bacc(trn_type="TRN2")
    lhs = nc.dram_tensor("lhs", [BPC, K, S * P], F8, kind="ExternalInput")
    tab = nc.dram_tensor("tab", [BPC, K, S * ts], F16, kind="ExternalInput")
    out = nc.dram_tensor("out", [BPC * ND], F32, kind="ExternalOutput")

    with tile.TileContext(nc) as tc:
        with (
            tc.tile_pool(name="consts", bufs=1) as consts,
            tc.tile_pool(name="mp", bufs=12) as mp,
            tc.tile_pool(name="qp", bufs=12) as qp,
            tc.tile_pool(name="mqp", bufs=12) as mqp,
            tc.tile_pool(name="resp", bufs=2) as resp,
            tc.tile_pool(name="psb", bufs=2, space="PSUM") as psb,
            tc.tile_pool(name="psa", bufs=4, space="PSUM") as psa,
        ):
            # variable-size units: small at batch start (fast pipeline fill)
            # and end (short drain chain), 12-slot in steady state
            UNITS = [US] * NU
            assert sum(UNITS) == S
            USTART = np.concatenate([[0], np.cumsum(UNITS)])
            NUV = len(UNITS)
            # chunks = ranges of units sharing one input tile pair
            CHUNKS = [(0, 2), (2, 4), (4, 6), (6, 10), (10, 14), (14, 18),
                      (18, NUV)]
            # stream both batches' inputs in chunks (separate tiles per chunk
            # so consumers only wait on their own slots)
            lhs_t = [[None] * NUV for _ in range(BPC)]
            tab_t = [[None] * NUV for _ in range(BPC)]
            for b in range(BPC):
                for ci, (ua, ub) in enumerate(CHUNKS):
                    s0, s1 = int(USTART[ua]), int(USTART[ub])
                    cs = s1 - s0
                    tab_sb = consts.tile([K, cs * ts], F16,
                                         tag=f"tab{b}c{ci}")
                    nc.sync.dma_start(
                        out=tab_sb[:, :], in_=tab[b, :, ts * s0 : ts * s1])
                    lhs_sb = consts.tile([K, cs * P], F8,
                                         tag=f"lhs{b}c{ci}")
                    nc.sync.dma_start(
                        out=lhs_sb[:, :], in_=lhs[b, :, P * s0 : P * s1])
                    for u in range(ua, ub):
                        off = int(USTART[u]) - s0
                        lhs_t[b][u] = (lhs_sb, off)
                        tab_t[b][u] = (tab_sb, off)

            for b in range(BPC):
                R = resp.tile([P, S], F32, tag="R")

                pending = None

                def flush_pending():
                    # previous pair's mult + per-slot reduces, once its sin
                    # is in flight on ACT
                    nonlocal pending
                    if pending is None:
                        return
                    u0, q2, aps2, uns = pending
                    for h in range(2):
                        u = u0 + h
                        un = uns[h]
                        us0 = int(USTART[u])
                        mq = mqp.tile([128, US * nt], F32, tag="mq")
                        nc.vector._custom_dve(
                            SEGSUM,
                            out=mq[:, 0 : un * nt].rearrange(
                                "p (s x) -> p s x", x=nt),
                            in0=q2[:, US * nt * h : US * nt * h + un * nt]
                            .rearrange("p (s x) -> p s x", x=nt),
                            in1=aps2[h][:, 0 : un * nt].rearrange(
                                "p (s x) -> p s x", x=nt),
                            s1=1.0,
                        )
                        nc.gpsimd.tensor_copy(
                            out=R[:, us0 : us0 + un].rearrange(
                                "p (s o) -> p s o", o=1),
                            in_=mq[:, 0 : un * nt].rearrange(
                                "p (s x) -> p s x", x=nt)[:, :, nt - 1 : nt],
                        )
                    pending = None

                for pu in range(NUV // 2):
                    # unit pair: shared 2-bank w-psum tile, per-unit A tiles
                    wps = psb.tile([128, 1024], F32, tag="wps")
                    aps2 = []
                    # all w-matmuls first: FRAC waits only these, so the
                    # A-matmuls run during frac/sin instead of gating it
                    for h in range(2):
                        u = 2 * pu + h
                        un = UNITS[u]
                        lhs_sb, lu = lhs_t[b][u]
                        tab_sb, tu = tab_t[b][u]
                        aps = psa.tile([128, 512], F32, tag="aps")
                        aps2.append(aps)
                        for j in range(un):
                            sl = lu + j           # slot within chunk tiles
                            nc.tensor.matmul(
                                out=wps[:, 512 * h + nt * j :
                                        512 * h + nt * (j + 1)],
                                lhsT=lhs_sb[:, P * sl : P * (sl + 1)],
                                rhs=tab_sb[:, ts * sl : ts * sl + nt],
                                start=True, stop=True,
                                tile_position=(0, 0),
                            )
                    for h in range(2):
                        u = 2 * pu + h
                        un = UNITS[u]
                        lhs_sb, lu = lhs_t[b][u]
                        tab_sb, tu = tab_t[b][u]
                        aps = aps2[h]
                        for j in range(un):
                            sl = lu + j
                            nc.tensor.matmul(
                                out=aps[:, nt * j : nt * (j + 1)],
                                lhsT=lhs_sb[:, P * sl : P * (sl + 1)],
                                rhs=tab_sb[:, ts * sl + nt : ts * (sl + 1)],
                                start=True, stop=True,
                                tile_position=(0, 0),
                            )
                    un0, un1 = UNITS[2 * pu], UNITS[2 * pu + 1]
                    m2 = mp.tile([128, 2 * US * nt], F16, tag="m2")
                    nc.vector._custom_dve(
                        FRAC1, out=m2[:, 0 : 2 * US * nt],
                        in0=wps[:, :].rearrange(
                            "p (h x) -> p h x", h=2)[:, :, 0 : US * nt],
                        s0=RND_MAGIC,
                    )
                    q2 = qp.tile([128, 2 * US * nt], F16, tag="q2")
                    for h in range(2):
                        nc.scalar.activation(
                            out=q2[:, US * nt * h : US * nt * (h + 1)],
                            in_=m2[:, US * nt * h : US * nt * (h + 1)],
                            func=AFT.Sin, scale=SIN_SCALE,
                        )
                    flush_pending()
                    pending = (2 * pu, q2, aps2, (un0, un1))
                    if pu == NUV // 2 - 2:
                        # pairs 0..pu-1 are flushed and final here
                        sfin = int(USTART[2 * pu])
                        nc.sync.dma_start(
                            out=out[b * ND : (b + 1) * ND].rearrange(
                                "(p s) -> p s", p=P)[:, 0:sfin],
                            in_=R[:, 0:sfin],
                        )
                flush_pending()

                sfin = int(USTART[2 * (NUV // 2 - 2)])
                nc.sync.dma_start(
                    out=out[b * ND : (b + 1) * ND].rearrange(
                        "(p s) -> p s", p=P)[:, sfin:S],
                    in_=R[:, sfin:S],
                )

    nc.compile()
    return nc


def _freqs(basis):
    half = DEG // 2
    return (
        np.concatenate(
            [
                np.cumsum(basis[:half]) - MAXB / 2,
                np.zeros(1, np.float32),
                np.cumsum(basis[half:]),
            ]
        ).astype(np.float64)
        * np.pi
    )


def _merge_terms(basis_x, basis_y):
    """Collapse the 81 (i,j) fourier terms to merged amplitude/phase terms.

    Returns (freqs_t [nt, 2] float64 in turns, M1 [nt, 81] complex,
    M2 [nt, 81] complex) with C'_t(bin) = sum_ij M1[t,ij] C_ij
    + M2[t,ij] conj(C_ij), C = Ps + i Pc.
    """
    xwt = _freqs(np.asarray(basis_x, np.float64)) / (2 * np.pi)  # turns
    ywt = _freqs(np.asarray(basis_y, np.float64)) / (2 * np.pi)

    def keyf(v):
        return round(float(v) * 2**20) / 2**20

    groups = {}
    for i in range(L):
        for j in range(L):
            f = (keyf(xwt[i]), keyf(ywt[j]))
            groups.setdefault(f, []).append(i * L + j)

    terms = []       # (f, list_plus, list_conj)
    used = set()
    for f in groups:
        if f in used:
            continue
        nf = (-f[0] if f[0] != 0 else 0.0, -f[1] if f[1] != 0 else 0.0)
        if f == nf:  # zero frequency
            terms.append((f, groups[f], []))
            used.add(f)
        elif nf in groups and nf not in used:
            # Im[C_f e^{ib}] + Im[C_-f e^{-ib}] = Im[(C_f - conj(C_-f)) e^{ib}]
            terms.append((f, groups[f], groups[nf]))
            used.add(f)
            used.add(nf)
        else:
            terms.append((f, groups[f], []))
            used.add(f)

    nt = len(terms)
    fr = np.zeros((nt, 2), np.float64)
    M1 = np.zeros((nt, L * L), np.complex128)
    M2 = np.zeros((nt, L * L), np.complex128)
    for t, (f, plus, conj) in enumerate(terms):
        fr[t] = f
        for k in plus:
            M1[t, k] += 1.0
        for k in conj:
            M2[t, k] -= 1.0
    return fr, M1, M2


def _split_f8(v, levels=4, step=16.0):
    """Exact-residual fp8 split: v ~= sum_l parts[l] / step**l."""
    parts = []
    r = np.asarray(v, np.float32)
    for _ in range(levels):
        p = r.astype(F8NP)
        parts.append(p)
        r = (r - p.astype(np.float32)) * np.float32(step)
    return parts


_CACHED_NC = {}


def _get_nc(nt=40):
    if nt not in _CACHED_NC:
        _CACHED_NC[nt] = build_bass(nt)
    return _CACHED_NC[nt]


def _prep(function_map, coord, basis_x, basis_y):
    """Host prep: term merge, per-batch sort, lhsT/table construction."""
    fr, M1, M2 = _merge_terms(basis_x, basis_y)
    nt = fr.shape[0]
    ts = 2 * nt

    fm = np.asarray(function_map, np.float32).reshape(B, NB, C)
    Cc = fm[..., 0:81] + 1j * fm[..., 81:162]          # [B, NB, 81]
    Ct = np.einsum("bnk,tk->bnt", Cc, M1) + np.einsum(
        "bnk,tk->bnt", np.conj(Cc), M2)                # [B, NB, nt]
    A = np.abs(Ct).astype(np.float16)                  # [B, NB, nt]
    phi = (np.angle(Ct) / TWO_PI).astype(np.float16)   # turns in [-0.5, 0.5]

    co = np.asarray(coord, np.float32)                 # [B, N, 2]
    gx, gy = co[..., 0], co[..., 1]
    hi = np.floor(gx / BAR)
    wi = np.floor(gy / BAR)
    lx = (gx - hi * BAR).astype(np.float32)
    ly = (gy - wi * BAR).astype(np.float32)
    bins = (hi * H + wi).astype(np.int32)              # [B, N]

    orders = np.empty((B, N), np.int64)
    lhs_all = np.empty((B, K, S * P), F8NP)
    tab_all = np.empty((B, K, S * ts), np.float16)

    # W rows: coord-split levels vs term freqs (exact f16 for 2^-k * quarter)
    wrows = np.zeros((KC, nt), np.float16)
    for lvl in range(4):
        wrows[lvl] = (fr[:, 0] / 16.0**lvl).astype(np.float16)
        wrows[4 + lvl] = (fr[:, 1] / 16.0**lvl).astype(np.float16)

    slotmaps = []
    for b in range(B):
        order = np.argsort(bins[b], kind="stable")
        orders[b] = order
        sb = bins[b][order]                            # sorted bins
        lxs = lx[b][order]
        lys = ly[b][order]

        # Greedy-pack sorted points into slots: each slot holds <=P points
        # drawn from <=KW distinct bins (any bins; the table is per-slot).
        # Works for any coord distribution, uniform or clumpy.
        uniq, counts = np.unique(sb, return_counts=True)
        slot_runs = []        # per slot: list of (bin, take)
        cur, cur_pts = [], 0
        for bn, cnt in zip(uniq, counts):
            rem = int(cnt)
            while rem > 0:
                if cur_pts == P or len(cur) == KW:
                    slot_runs.append(cur)
                    cur, cur_pts = [], 0
                take = min(rem, P - cur_pts)
                cur.append((int(bn), take))
                cur_pts += take
                rem -= take
        if cur:
            slot_runs.append(cur)
        assert len(slot_runs) <= S, (
            f"batch {b}: needs {len(slot_runs)} slots > {S}")

        # per sorted point: slot, lane, local one-hot row
        nslot = len(slot_runs)
        slot_of = np.empty(N, np.int64)
        lane_of = np.empty(N, np.int64)
        row_of = np.empty(N, np.int64)
        slot_bins = []        # per slot: bin list
        p0 = 0
        for s, runs in enumerate(slot_runs):
            n_s = sum(t for _, t in runs)
            sl = slice(p0, p0 + n_s)
            slot_of[sl] = s
            lane_of[sl] = np.arange(n_s)
            row_of[sl] = np.repeat(np.arange(len(runs)),
                                   [t for _, t in runs])
            slot_bins.append(np.array([bn for bn, _ in runs], np.int64))
            p0 += n_s
        assert p0 == N
        slotmaps.append(slot_of * P + lane_of)

        # lhsT: [K, S*P]; rows 0..7 coord splits, rows 8..71 one-hot.
        # Unused lanes / slots stay all-zero (w=0, A=0, ignored on host).
        pos = slot_of * P + lane_of                    # [N] device position
        lhsb = np.zeros((K, ND), np.float32)
        xs = _split_f8(lxs)
        ys = _split_f8(lys)
        for lvl in range(4):
            lhsb[lvl, pos] = xs[lvl].astype(np.float32)
            lhsb[4 + lvl, pos] = ys[lvl].astype(np.float32)
        lhsb[KC + row_of, pos] = 1.0
        lhs_all[b] = lhsb.astype(F8NP)

        # table: per slot its own bins' [phi | A] cols, W rows on top
        tb = np.zeros((K, S, ts), np.float16)
        tb[0:KC, 0:nslot, 0:nt] = wrows[:, None, :]
        for s, bl in enumerate(slot_bins):
            tb[KC : KC + len(bl), s, 0:nt] = phi[b][bl]
            tb[KC : KC + len(bl), s, nt:ts] = A[b][bl]
        tab_all[b] = np.ascontiguousarray(tb.reshape(K, S * ts))

    return nt, lhs_all, tab_all, orders, slotmaps


def run(function_map, coord, basis_x, basis_y, **spmd_kwargs):
    nt, lhs_all, tab_all, orders, slotmaps = _prep(
        function_map, coord, basis_x, basis_y)
    in_maps = []
    for c in range(NCORES):
        sl = slice(BPC * c, BPC * (c + 1))
        in_maps.append({
            "lhs": np.ascontiguousarray(lhs_all[sl]),
            "tab": np.ascontiguousarray(tab_all[sl]),
        })
    res = run_bass_kernel_spmd(
        _get_nc(nt), in_maps, core_ids=list(range(NCORES)), **spmd_kwargs
    )
    out = np.empty((B, N), np.float32)
    for c in range(NCORES):
        ro = res.results[c]["out"].reshape(BPC, P, S)
        for bb in range(BPC):
            b = BPC * c + bb
            # device position pos = slot*P + lane maps to R[lane, slot]
            flat = ro[bb].T.reshape(ND)                # [slot, lane] flat
            out[b, orders[b]] = flat[slotmaps[b]]
    return out.reshape(B, N, 1), res


def kernel(function_map, coord, basis_x, basis_y):
    out, _ = run(function_map, coord, basis_x, basis_y)
    return out


# revision 52
# speedup vs baseline: 1.0065x; 1.0065x over previous
"""Trainium2 Bass kernel for nn_NExpR_14903536517949 (embedding_lookup).

Reference computation per query point (b, n):
    hi = floor(gx/2), wi = floor(gy/2)                 (bin indices, 64x64 grid)
    params = function_map[b, hi, wi]                   (162 channels: Ps|Pc)
    lx = gx mod 2, ly = gy mod 2                       (local coords)
    out = sum_ij Ps_ij sin(lx xw_i + ly yw_j) + Pc_ij cos(lx xw_i + ly yw_j)

Host-side algebraic transforms:
  * out = Im sum_ij C_ij e^{i b_ij}, C = Ps + i Pc, b_ij = 2pi(fx_i lx + fy_j ly)
    (freqs in turns). Terms sharing a frequency pair merge; Hermitian pairs
    (f, -f) merge via C' = C_f - conj(C_{-f}). For the spec input (uniform
    basis 0.5 -> quarter-integer freqs with a doubled zero row) 81 terms
    collapse to NT = 40 amplitude/phase pairs: out = sum_t A_t sin(2pi(fx_t lx
    + fy_t ly + phi_t)).
  * Points are sorted by bin per batch and greedy-packed into 240 slots of
    <=128 points drawing from <=64 distinct bins each (the per-slot table
    carries exactly those bins' rows), so each point's bin row is fetched by
    a one-hot matmul instead of a per-point DMA gather. Output is
    un-permuted on host. Works for any coord distribution.

Device pipeline per slot (128 points), all data delivered by dense DMA
(no per-point gather):
  * PE: two matmuls with a shared fp8 lhsT [72, 128] = [8 coord-split rows |
    64 one-hot window rows] against the slot's f16 table column block:
      w-psum = basis + phi   (coord rows x freq rows + one-hot x phi rows)
      A-psum = amplitudes    (one-hot x A rows)
  * DVE: FRAC1 custom op m = w - round(w) (fp32 magic) over a PAIR of
    12-slot units sharing a 2-bank w-psum tile, fp16 out; SEGSUM custom op
    per-slot prefix sums of q*A against the unit's A-psum (totals at col
    NT-1). DVE is the bottleneck engine (~79% busy); segsums lag one pair
    behind sin so DVE never waits on ACT.
  * ACT: q = sin(SIN_SCALE * m) per pair.
  * Pool: copy per-slot totals into the result tile.
  * Inputs stream in unit-range chunks so compute starts ~5us in; outputs
    drain in two DMAs per batch.

Distribution: data-parallel over batch, 2 images per core, 8 cores.
"""

import math

import numpy as np
import ml_dtypes

import concourse.bass as bass
import concourse.mybir as mybir
import concourse.tile as tile
from concourse import bacc
from concourse.bass_utils import run_bass_kernel_spmd

import concourse.dve_ops as dve_ops
from concourse.dve_spec import C0, Spec, Src0, Src1, lower
from concourse.dve_uop import DveOpSpec

F32 = mybir.dt.float32
F16 = mybir.dt.float16
F8 = mybir.dt.float8e4
ALU = mybir.AluOpType
AFT = mybir.ActivationFunctionType

# Problem shape (hardcoded per spec)
B, H, W, C = 16, 64, 64, 162
N = 30000
NCORES = 8
BPC = B // NCORES            # batches per core = 2
DEG, MAXB, BAR = 8, 4.0, 2.0
L = DEG + 1                  # 9
NB = H * W                   # bins per batch = 4096
TWO_PI = 2.0 * math.pi
SIN_SCALE = 6.2831820        # slightly under 2*pi: |m*scale| < pi at m=+-0.5
RND_MAGIC = 1.5 * 2.0**23    # fp32 add-sub round-to-nearest trick

# Kernel layout constants
P = 128                      # points per slot (partitions)
S = 240                      # slots per batch
ND = P * S                   # 30720 padded points per batch
KC = 8                       # coord-split lhsT rows (4 per axis, x16 scales)
KW = 64                      # one-hot window rows (bins per window)
K = KC + KW                  # 72
US = 12                      # slots per pipeline unit (psum bank = 512 f32)
NU = S // US                 # 20 units per batch
F8NP = ml_dtypes.float8_e4m3fn


def _frac1_ref(in0, in1, s0, s1=0.0, imm2=0.0):
    w = np.asarray(in0, np.float32)
    r = (w + np.float32(s0)) - np.float32(s0)
    return w - r


def _register_frac1():
    """Custom DVE op: out = w - round(w) (fp32 magic-number rounding)."""
    if "FRAC1_ANT" in dve_ops._SUB_OPCODE_FOR_NAME:
        return next(op for op in dve_ops.OPS if op.name == "FRAC1_ANT")
    w = Src0
    spec = Spec(body=w - ((w + C0) - C0), reference=_frac1_ref)
    shas = {}
    for ver in ("v3", "v4"):
        d = DveOpSpec(name="FRAC1_ANT", opcode=0, uops=lower(spec, ver=ver),
                      rd1_en=False)
        shas[ver] = d.sha(ver)
    op = dve_ops.DveOp("FRAC1_ANT", spec, subdim=False, uops_sha=shas)
    dve_ops.OPS.append(op)
    dve_ops._SUB_OPCODE_FOR_NAME[op.name] = (
        dve_ops._CUSTOM_DVE_ROW_BASE + len(dve_ops.OPS) - 1
    )
    dve_ops.CUSTOM_DVE_SPECS[op.name] = op.spec
    return op


def _segsum_ref(in0, in1, s0, s1=0.0, imm2=0.0):
    a = np.asarray(in0, np.float32)
    b = np.asarray(in1, np.float32)
    return np.cumsum(a * b, axis=-1) * np.float32(s1)


def _register_segsum():
    """Hand-built 3-state uop FSM: per-page running prefix sums of
    Src0*Src1*s1 (page = innermost dim); each page's total lands at its
    last column."""
    import dataclasses as _dc
    from concourse.dve_uop import Trigger, OutPath, OutSel, AluInp, AluOp

    if "SEGSUM_ANT" in dve_ops._SUB_OPCODE_FOR_NAME:
        return next(op for op in dve_ops.OPS if op.name == "SEGSUM_ANT")

    def _build(ver):
        from concourse.dve_ops import TENSOR_TENSOR_REDUCE as TTR
        u0, u1 = [_dc.replace(u) for u in lower(TTR.spec, ver=ver)]
        acc_stage = next(
            i for i, dp in enumerate(u1.datapath_config)
            if dp.op == AluOp.ADD and dp.alu_src0 == AluInp.CURR_ALU_OUT
        )
        dp_seed = list(u0.datapath_config)
        dp_seed[acc_stage] = _dc.replace(
            dp_seed[acc_stage], op=AluOp.BYPASS,
            alu_src0=AluInp.PREV_ALU_OUT, alu_src1=AluInp.PREV_ALU_OUT,
        )
        wr = {**u1.out, OutPath.WR0_LO: OutSel.ALU_OUT,
              OutPath.WR0_HI: OutSel.ALU_OUT}
        wren = {**{p: 0 for p in OutPath}, OutPath.WR0_LO: 1,
                OutPath.WR0_HI: 1}
        init = _dc.replace(
            u0, datapath_config=dp_seed, out=wr, out_enable=wren,
            require_inp0=1, require_inp1=1,
            trigger=(Trigger.COUNT, Trigger.NONE, Trigger.NONE),
            next_uop=(1, 0, 0), repeat_count=1,
        )
        steady = _dc.replace(
            u1, out=wr, out_enable=wren,
            trigger=(Trigger.SRC_TENSOR_DONE, Trigger.SUB_DIM_DONE,
                     Trigger.NONE),
            next_uop=(0, 2, 0),
        )
        pageseed = _dc.replace(
            u0, datapath_config=dp_seed, out=wr, out_enable=wren,
            require_inp0=1, require_inp1=1,
            trigger=(Trigger.SRC_TENSOR_DONE, Trigger.SUB_DIM_DONE,
                     Trigger.COUNT),
            next_uop=(0, 2, 1), repeat_count=1,
        )
        for u in (init, steady, pageseed):
            u.validate(ver)
        return [init, steady, pageseed]

    class _HandDveOp(dve_ops.DveOp):
        def compile(self, ver):
            key = (self.name, ver)
            if (r := dve_ops._COMPILE_CACHE.get(key)) is not None:
                return r
            result = DveOpSpec(
                name=self.name,
                opcode=dve_ops.get_dve_sub_opcode(self.name),
                uops=_build(ver), rd1_en=True,
            )
            dve_ops._COMPILE_CACHE[key] = result
            return result

    spec = Spec(body=Src0 * Src1, reference=_segsum_ref)
    op = _HandDveOp("SEGSUM_ANT", spec, subdim=True, uops_sha={})
    dve_ops.OPS.append(op)
    dve_ops._SUB_OPCODE_FOR_NAME[op.name] = (
        dve_ops._CUSTOM_DVE_ROW_BASE + len(dve_ops.OPS) - 1
    )
    dve_ops.CUSTOM_DVE_SPECS[op.name] = op.spec
    return op


FRAC1 = _register_frac1()
SEGSUM = _register_segsum()


def build_bass(nt):
    """nt = number of merged terms (compile-time)."""
    assert US * nt <= 512, f"unit exceeds psum bank: {nt=}"
    ts = 2 * nt                  # table cols per window: [phi | A]
    nc = bacc.Bacc(trn_type="TRN2")
    lhs = nc.dram_tensor("lhs", [BPC, K, S * P], F8, kind="ExternalInput")
    tab = nc.dram_tensor("tab", [BPC, K, S * ts], F16, kind="ExternalInput")
    out = nc.dram_tensor("out", [BPC * ND], F32, kind="ExternalOutput")

    with tile.TileContext(nc) as tc:
        with (
            tc.tile_pool(name="consts", bufs=1) as consts,
            tc.tile_pool(name="mp", bufs=12) as mp,
            tc.tile_pool(name="qp", bufs=12) as qp,
            tc.tile_pool(name="mqp", bufs=12) as mqp,
            tc.tile_pool(name="resp", bufs=2) as resp,
            tc.tile_pool(name="psb", bufs=2, space="PSUM") as psb,
            tc.tile_pool(name="psa", bufs=4, space="PSUM") as psa,
        ):
            # variable-size units: small at batch start (fast pipeline fill)
            # and end (short drain chain), 12-slot in steady state
            UNITS = [US] * NU
            assert sum(UNITS) == S
            USTART = np.concatenate([[0], np.cumsum(UNITS)])
            NUV = len(UNITS)
            # chunks = ranges of units sharing one input tile pair
            CHUNKS = [(0, 2), (2, 4), (4, 8), (8, 12),
                      (12, NUV)]
            # stream both batches' inputs in chunks (separate tiles per chunk
            # so consumers only wait on their own slots)
            lhs_t = [[None] * NUV for _ in range(BPC)]
            tab_t = [[None] * NUV for _ in range(BPC)]
            for b in range(BPC):
                for ci, (ua, ub) in enumerate(CHUNKS):
                    s0, s1 = int(USTART[ua]), int(USTART[ub])
                    cs = s1 - s0
                    tab_sb = consts.tile([K, cs * ts], F16,
                                         tag=f"tab{b}c{ci}")
                    nc.sync.dma_start(
                        out=tab_sb[:, :], in_=tab[b, :, ts * s0 : ts * s1])
                    lhs_sb = consts.tile([K, cs * P], F8,
                                         tag=f"lhs{b}c{ci}")
                    nc.sync.dma_start(
                        out=lhs_sb[:, :], in_=lhs[b, :, P * s0 : P * s1])
                    for u in range(ua, ub):
                        off = int(USTART[u]) - s0
                        lhs_t[b][u] = (lhs_sb, off)
                        tab_t[b][u] = (tab_sb, off)

            for b in range(BPC):
                R = resp.tile([P, S], F32, tag="R")

                pending = None

                def flush_pending():
                    # previous pair's mult + per-slot reduces, once its sin
                    # is in flight on ACT
                    nonlocal pending
                    if pending is None:
                        return
                    u0, q2, aps2, uns = pending
                    for h in range(2):
                        u = u0 + h
                        un = uns[h]
                        us0 = int(USTART[u])
                        mq = mqp.tile([128, US * nt], F32, tag="mq")
                        nc.vector._custom_dve(
                            SEGSUM,
                            out=mq[:, 0 : un * nt].rearrange(
                                "p (s x) -> p s x", x=nt),
                            in0=q2[:, US * nt * h : US * nt * h + un * nt]
                            .rearrange("p (s x) -> p s x", x=nt),
                            in1=aps2[h][:, 0 : un * nt].rearrange(
                                "p (s x) -> p s x", x=nt),
                            s1=1.0,
                        )
                        nc.gpsimd.tensor_copy(
                            out=R[:, us0 : us0 + un].rearrange(
                                "p (s o) -> p s o", o=1),
                            in_=mq[:, 0 : un * nt].rearrange(
                                "p (s x) -> p s x", x=nt)[:, :, nt - 1 : nt],
                        )
                    pending = None

                for pu in range(NUV // 2):
                    # unit pair: shared 2-bank w-psum tile, per-unit A tiles
                    wps = psb.tile([128, 1024], F32, tag="wps")
                    aps2 = []
                    # all w-matmuls first: FRAC waits only these, so the
                    # A-matmuls run during frac/sin instead of gating it
                    for h in range(2):
                        u = 2 * pu + h
                        un = UNITS[u]
                        lhs_sb, lu = lhs_t[b][u]
                        tab_sb, tu = tab_t[b][u]
                        aps = psa.tile([128, 512], F32, tag="aps")
                        aps2.append(aps)
                        for j in range(un):
                            sl = lu + j           # slot within chunk tiles
                            nc.tensor.matmul(
                                out=wps[:, 512 * h + nt * j :
                                        512 * h + nt * (j + 1)],
                                lhsT=lhs_sb[:, P * sl : P * (sl + 1)],
                                rhs=tab_sb[:, ts * sl : ts * sl + nt],
                                start=True, stop=True,
                                tile_position=(0, 0),
                            )
                    for h in range(2):
                        u = 2 * pu + h
                        un = UNITS[u]
                        lhs_sb, lu = lhs_t[b][u]
                        tab_sb, tu = tab_t[b][u]
                        aps = aps2[h]
                        for j in range(un):
                            sl = lu + j
                            nc.tensor.matmul(
                                out=aps[:, nt * j : nt * (j + 1)],
                                lhsT=lhs_sb[:, P * sl : P * (sl + 1)],
                                rhs=tab_sb[:, ts * sl + nt : ts * (sl + 1)],
                                start=True, stop=True,
                                tile_position=(0, 0),
                            )
                    un0, un1 = UNITS[2 * pu], UNITS[2 * pu + 1]
                    m2 = mp.tile([128, 2 * US * nt], F16, tag="m2")
                    nc.vector._custom_dve(
                        FRAC1, out=m2[:, 0 : 2 * US * nt],
                        in0=wps[:, :].rearrange(
                            "p (h x) -> p h x", h=2)[:, :, 0 : US * nt],
                        s0=RND_MAGIC,
                    )
                    q2 = qp.tile([128, 2 * US * nt], F16, tag="q2")
                    for h in range(2):
                        nc.scalar.activation(
                            out=q2[:, US * nt * h : US * nt * (h + 1)],
                            in_=m2[:, US * nt * h : US * nt * (h + 1)],
                            func=AFT.Sin, scale=SIN_SCALE,
                        )
                    flush_pending()
                    pending = (2 * pu, q2, aps2, (un0, un1))
                    if pu == NUV // 2 - 2:
                        # pairs 0..pu-1 are flushed and final here
                        sfin = int(USTART[2 * pu])
                        nc.sync.dma_start(
                            out=out[b * ND : (b + 1) * ND].rearrange(
                                "(p s) -> p s", p=P)[:, 0:sfin],
                            in_=R[:, 0:sfin],
                        )
                flush_pending()

                sfin = int(USTART[2 * (NUV // 2 - 2)])
                nc.sync.dma_start(
                    out=out[b * ND : (b + 1) * ND].rearrange(
                        "(p s) -> p s", p=P)[:, sfin:S],
                    in_=R[:, sfin:S],
                )

    nc.compile()
    return nc


def _freqs(basis):
    half = DEG // 2
    return (
        np.concatenate(
            [
                np.cumsum(basis[:half]) - MAXB / 2,
                np.zeros(1, np.float32),
                np.cumsum(basis[half:]),
            ]
        ).astype(np.float64)
        * np.pi
    )


def _merge_terms(basis_x, basis_y):
    """Collapse the 81 (i,j) fourier terms to merged amplitude/phase terms.

    Returns (freqs_t [nt, 2] float64 in turns, M1 [nt, 81] complex,
    M2 [nt, 81] complex) with C'_t(bin) = sum_ij M1[t,ij] C_ij
    + M2[t,ij] conj(C_ij), C = Ps + i Pc.
    """
    xwt = _freqs(np.asarray(basis_x, np.float64)) / (2 * np.pi)  # turns
    ywt = _freqs(np.asarray(basis_y, np.float64)) / (2 * np.pi)

    def keyf(v):
        return round(float(v) * 2**20) / 2**20

    groups = {}
    for i in range(L):
        for j in range(L):
            f = (keyf(xwt[i]), keyf(ywt[j]))
            groups.setdefault(f, []).append(i * L + j)

    terms = []       # (f, list_plus, list_conj)
    used = set()
    for f in groups:
        if f in used:
            continue
        nf = (-f[0] if f[0] != 0 else 0.0, -f[1] if f[1] != 0 else 0.0)
        if f == nf:  # zero frequency
            terms.append((f, groups[f], []))
            used.add(f)
        elif nf in groups and nf not in used:
            # Im[C_f e^{ib}] + Im[C_-f e^{-ib}] = Im[(C_f - conj(C_-f)) e^{ib}]
            terms.append((f, groups[f], groups[nf]))
            used.add(f)
            used.add(nf)
        else:
            terms.append((f, groups[f], []))
            used.add(f)

    nt = len(terms)
    fr = np.zeros((nt, 2), np.float64)
    M1 = np.zeros((nt, L * L), np.complex128)
    M2 = np.zeros((nt, L * L), np.complex128)
    for t, (f, plus, conj) in enumerate(terms):
        fr[t] = f
        for k in plus:
            M1[t, k] += 1.0
        for k in conj:
            M2[t, k] -= 1.0
    return fr, M1, M2


def _split_f8(v, levels=4, step=16.0):
    """Exact-residual fp8 split: v ~= sum_l parts[l] / step**l."""
    parts = []
    r = np.asarray(v, np.float32)
    for _ in range(levels):
        p = r.astype(F8NP)
        parts.append(p)
        r = (r - p.astype(np.float32)) * np.float32(step)
    return parts


_CACHED_NC = {}


def _get_nc(nt=40):
    if nt not in _CACHED_NC:
        _CACHED_NC[nt] = build_bass(nt)
    return _CACHED_NC[nt]


def _prep(function_map, coord, basis_x, basis_y):
    """Host prep: term merge, per-batch sort, lhsT/table construction."""
    fr, M1, M2 = _merge_terms(basis_x, basis_y)
    nt = fr.shape[0]
    ts = 2 * nt

    fm = np.asarray(function_map, np.float32).reshape(B, NB, C)
    Cc = fm[..., 0:81] + 1j * fm[..., 81:162]          # [B, NB, 81]
    Ct = np.einsum("bnk,tk->bnt", Cc, M1) + np.einsum(
        "bnk,tk->bnt", np.conj(Cc), M2)                # [B, NB, nt]
    A = np.abs(Ct).astype(np.float16)                  # [B, NB, nt]
    phi = (np.angle(Ct) / TWO_PI).astype(np.float16)   # turns in [-0.5, 0.5]

    co = np.asarray(coord, np.float32)                 # [B, N, 2]
    gx, gy = co[..., 0], co[..., 1]
    hi = np.floor(gx / BAR)
    wi = np.floor(gy / BAR)
    lx = (gx - hi * BAR).astype(np.float32)
    ly = (gy - wi * BAR).astype(np.float32)
    bins = (hi * H + wi).astype(np.int32)              # [B, N]

    orders = np.empty((B, N), np.int64)
    lhs_all = np.empty((B, K, S * P), F8NP)
    tab_all = np.empty((B, K, S * ts), np.float16)

    # W rows: coord-split levels vs term freqs (exact f16 for 2^-k * quarter)
    wrows = np.zeros((KC, nt), np.float16)
    for lvl in range(4):
        wrows[lvl] = (fr[:, 0] / 16.0**lvl).astype(np.float16)
        wrows[4 + lvl] = (fr[:, 1] / 16.0**lvl).astype(np.float16)

    slotmaps = []
    for b in range(B):
        order = np.argsort(bins[b], kind="stable")
        orders[b] = order
        sb = bins[b][order]                            # sorted bins
        lxs = lx[b][order]
        lys = ly[b][order]

        # Greedy-pack sorted points into slots: each slot holds <=P points
        # drawn from <=KW distinct bins (any bins; the table is per-slot).
        # Works for any coord distribution, uniform or clumpy.
        uniq, counts = np.unique(sb, return_counts=True)
        slot_runs = []        # per slot: list of (bin, take)
        cur, cur_pts = [], 0
        for bn, cnt in zip(uniq, counts):
            rem = int(cnt)
            while rem > 0:
                if cur_pts == P or len(cur) == KW:
                    slot_runs.append(cur)
                    cur, cur_pts = [], 0
                take = min(rem, P - cur_pts)
                cur.append((int(bn), take))
                cur_pts += take
                rem -= take
        if cur:
            slot_runs.append(cur)
        assert len(slot_runs) <= S, (
            f"batch {b}: needs {len(slot_runs)} slots > {S}")

        # per sorted point: slot, lane, local one-hot row
        nslot = len(slot_runs)
        slot_of = np.empty(N, np.int64)
        lane_of = np.empty(N, np.int64)
        row_of = np.empty(N, np.int64)
        slot_bins = []        # per slot: bin list
        p0 = 0
        for s, runs in enumerate(slot_runs):
            n_s = sum(t for _, t in runs)
            sl = slice(p0, p0 + n_s)
            slot_of[sl] = s
            lane_of[sl] = np.arange(n_s)
            row_of[sl] = np.repeat(np.arange(len(runs)),
                                   [t for _, t in runs])
            slot_bins.append(np.array([bn for bn, _ in runs], np.int64))
            p0 += n_s
        assert p0 == N
        slotmaps.append(slot_of * P + lane_of)

        # lhsT: [K, S*P]; rows 0..7 coord splits, rows 8..71 one-hot.
        # Unused lanes / slots stay all-zero (w=0, A=0, ignored on host).
        pos = slot_of * P + lane_of                    # [N] device position
        lhsb = np.zeros((K, ND), np.float32)
        xs = _split_f8(lxs)
        ys = _split_f8(lys)
        for lvl in range(4):
            lhsb[lvl, pos] = xs[lvl].astype(np.float32)
            lhsb[4 + lvl, pos] = ys[lvl].astype(np.float32)
        lhsb[KC + row_of, pos] = 1.0
        lhs_all[b] = lhsb.astype(F8NP)

        # table: per slot its own bins' [phi | A] cols, W rows on top
        tb = np.zeros((K, S, ts), np.float16)
        tb[0:KC, 0:nslot, 0:nt] = wrows[:, None, :]
        for s, bl in enumerate(slot_bins):
            tb[KC : KC + len(bl), s, 0:nt] = phi[b][bl]
            tb[KC : KC + len(bl), s, nt:ts] = A[b][bl]
        tab_all[b] = np.ascontiguousarray(tb.reshape(K, S * ts))

    return nt, lhs_all, tab_all, orders, slotmaps


def run(function_map, coord, basis_x, basis_y, **spmd_kwargs):
    nt, lhs_all, tab_all, orders, slotmaps = _prep(
        function_map, coord, basis_x, basis_y)
    in_maps = []
    for c in range(NCORES):
        sl = slice(BPC * c, BPC * (c + 1))
        in_maps.append({
            "lhs": np.ascontiguousarray(lhs_all[sl]),
            "tab": np.ascontiguousarray(tab_all[sl]),
        })
    res = run_bass_kernel_spmd(
        _get_nc(nt), in_maps, core_ids=list(range(NCORES)), **spmd_kwargs
    )
    out = np.empty((B, N), np.float32)
    for c in range(NCORES):
        ro = res.results[c]["out"].reshape(BPC, P, S)
        for bb in range(BPC):
            b = BPC * c + bb
            # device position pos = slot*P + lane maps to R[lane, slot]
            flat = ro[bb].T.reshape(ND)                # [slot, lane] flat
            out[b, orders[b]] = flat[slotmaps[b]]
    return out.reshape(B, N, 1), res


def kernel(function_map, coord, basis_x, basis_y):
    out, _ = run(function_map, coord, basis_x, basis_y)
    return out
